# revision 28
# baseline (speedup 1.0000x reference)
"""MoE FFN (shared + top-2 routed experts) on 8 Trainium2 NeuronCores.

Strategy (expert-parallel with host-side token dispatch):
  - Router (logits -> top-2 -> softmax) is computed on the host in float64;
    it is a tiny [T,768]@[768,8] matmul. The resulting combine weights and
    per-expert token lists define the dispatch.
  - Core c processes: (a) the tokens routed to expert c (gathered, padded to
    C_pad), through expert c's SwiGLU weights, scaled by combine weight;
    (b) a 512-token slice of all tokens through the shared expert.
  - Each core returns [C_pad + 512, 768] fp32 partial outputs; the host
    scatter-adds routed partials and adds the shared slices.

Device kernel (per core, SPMD — identical program, different data):
  The PE issue stream is the floor (~148us of matmuls at N/2.4GHz+2.5ns
  each); everything else is engineered so the PE never waits:
  - Gate/up weights are interleaved per 128-col f-chunk ([P, f, g|u, k, 128])
    so the critical set before the first real matmul is only ~1.2 MB
    (xt of supertile 0 + f-chunk 0), spread critical-first across the
    three DMA queues (sync/scalar HWDGE + gpsimd SWDGE). Supertile 0's
    mm1 consumes one 0.4MB f-chunk per ~2.6us; later supertiles reuse
    resident weights.
  - A short PE warm-up (zeros matmuls) spans the DMA window so the HAM
    clock gate is at 2.4GHz when real work starts (~12us).
  - mm1 computes H^T = (W.T)(X^T) per 128-wide F chunk accumulating over
    D in PSUM (fp32), ACT applies SiLU, DVE multiplies gate*up -> bf16.
    mm2 contracts over F with H^T chunks stationary, producing
    [128 rows, 768] fp32 in two 384-col PSUM banks; po0's accumulation
    completes before po1's starts and each half is scaled (per-row
    combine weight via per-partition tensor_scalar) and stored
    separately, so the final store overlaps the last matmul group.
  - The ragged supertile (rows beyond the last full 512) runs last to
    keep the post-matmul tail minimal.
"""

import numpy as np
import ml_dtypes

import concourse.bass as bass  # noqa: F401
import concourse.mybir as mybir
import concourse.tile as tile
from concourse import bacc
from concourse.bass_utils import run_bass_kernel_spmd

BF16 = ml_dtypes.bfloat16

D = 768
F = 1536
E = 8
K_ACTIVE = 2
NCORES = 8
P = 128
KD = D // P   # 6 contraction chunks for mm1
KF = F // P   # 12 contraction chunks for mm2 == f-chunks of mm1
N_WARM = 17   # PE warm-up matmuls. HAM empirically needs ~5us of
              # continuous PE-busy to un-throttle (not the nominal
              # 3.4us window), so the warm-up starts at ~6.4us (memset
              # on the early-free gpsimd engine, not vector) and runs
              # ~12 cold + ~5 warm matmuls, ending just as the critical
              # input DMAs land (~13us).

_prog_cache = {}


def _route_host(xf, w_router, expert_bias):
    """Top-2 routing in float64 (matches the fp32 jax reference to ~1e-7)."""
    T = xf.shape[0]
    logits = xf.astype(np.float64) @ w_router.astype(np.float64)
    logits += expert_bias.astype(np.float64)[None, :]
    ar = np.arange(T)
    i1 = np.argmax(logits, axis=1)
    v1 = logits[ar, i1]
    l2 = logits.copy()
    l2[ar, i1] = -np.inf
    i2 = np.argmax(l2, axis=1)
    v2 = logits[ar, i2]
    e2 = np.exp(v2 - v1)
    s = 1.0 + e2
    w1 = (1.0 / s).astype(np.float32)
    w2 = (e2 / s).astype(np.float32)
    return i1, i2, w1, w2


def _supertiles(c_pad, r_total):
    """Split rows into (start, size, weight_set) chunks of <=512 rows."""
    out = []
    r0 = 0
    while r0 < c_pad:
        ns = min(512, c_pad - r0)
        out.append((r0, ns, 0))
        r0 += ns
    while r0 < r_total:
        ns = min(512, r_total - r0)
        out.append((r0, ns, 1))
        r0 += ns
    return out


def _shuf_gu(wg, wu):
    """Two [D, F] mats -> [128, KF*2*KD*128]: partition-major, f-chunk-major
    interleaved layout; element [p, f, t, k, c] = W_t[k*128+p, f*128+c]."""
    def one(w):
        return w.reshape(KD, P, KF, P).transpose(1, 2, 0, 3)
    return np.ascontiguousarray(
        np.stack([one(wg), one(wu)], axis=2).reshape(P, KF * 2 * KD * P))


def _shuf_wd(w):
    """[F, D] -> [128, KF*D]: partition-major layout."""
    return np.ascontiguousarray(
        w.reshape(KF, P, D).transpose(1, 0, 2).reshape(P, KF * D))


def _shuf_xt(xt, sts):
    """[D, R] -> [128, KD*R] with per-supertile blocks."""
    out = np.empty((P, KD * xt.shape[1]), xt.dtype)
    for (r0, ns, _) in sts:
        blk = xt[:, r0:r0 + ns].reshape(KD, P, ns).transpose(1, 0, 2)
        out[:, KD * r0:KD * (r0 + ns)] = blk.reshape(P, KD * ns)
    return np.ascontiguousarray(out)


def _build_program(r_total, c_pad, c_used):
    dt = mybir.dt
    nc = bacc.Bacc("TRN2", target_bir_lowering=False, debug=False)
    xt_d = nc.dram_tensor("xt", [P, KD * r_total], dt.bfloat16,
                          kind="ExternalInput")
    gu_d = [nc.dram_tensor(f"gu{s}", [P, KF * 2 * KD * P], dt.bfloat16,
                           kind="ExternalInput") for s in range(2)]
    wd_d = [nc.dram_tensor(f"wd{s}", [P, KF * D], dt.bfloat16,
                           kind="ExternalInput") for s in range(2)]
    n_tiles = r_total // P
    sc_d = nc.dram_tensor("scale", [P, n_tiles], dt.float32,
                          kind="ExternalInput")
    out_d = nc.dram_tensor("out", [r_total, D], dt.float32,
                           kind="ExternalOutput")

    silu = mybir.ActivationFunctionType.Silu
    # (row0, layout rows, compute rows, weight set): pad rows beyond the
    # actual max routed count exist in the layout but are never computed.
    sts = []
    for (r0, ns, s) in _supertiles(c_pad, r_total):
        nc_rows = ns if s == 1 else min(ns, max(0, c_used - r0))
        if nc_rows > 0:
            sts.append((r0, ns, nc_rows, s))

    FC = 2 * KD * P  # elements per f-chunk of a gu tensor (per partition)

    with tile.TileContext(nc) as tc:
        with (
            tc.tile_pool(name="const", bufs=1) as const,
            tc.tile_pool(name="work", bufs=3) as work,
            tc.tile_pool(name="outp", bufs=8) as outp,
            tc.tile_pool(name="ps1", bufs=2, space="PSUM") as ps1,
            tc.tile_pool(name="ps2", bufs=2, space="PSUM") as ps2,
        ):
            sc_sb = const.tile([P, n_tiles], dt.float32, tag="sc")
            xt_sb = const.tile([P, KD * r_total], dt.bfloat16, tag="xt")
            gu_sb, wd_sb = [], []
            for s in range(2):
                gu_t = const.tile([P, KF, 2, KD, P], dt.bfloat16,
                                  tag=f"gu{s}")
                wd_t = const.tile([P, KF, D], dt.bfloat16, tag=f"wd{s}")
                gu_sb.append(gu_t)
                wd_sb.append(wd_t)

            # DMA plan, critical-first. The first real matmul needs
            # xt(supertile 0) + gu0 f-chunk 0 (~1.2 MB); each further
            # f-chunk (0.4 MB) is consumed one per ~2.6us. Queues drain
            # FIFO per engine with round-robin across queues (~1/3 of
            # ~320 GB/s each while all are busy), so the critical pieces
            # lead all three queues and f-chunks rotate across them.
            r0_, ns0 = sts[0][0], sts[0][1]
            h0 = KD * ns0 // 2

            # Warm-up source tile, zeroed on gpsimd BEFORE its dma_start
            # instructions (each ~1us of SWDGE emission) so the PE can
            # begin warming at ~6.4us; a vector-engine memset would wait
            # out the longer vector preamble and delay warm-up to ~8.4us.
            wsrc = const.tile([P, 512], dt.bfloat16, tag="wsrc")
            nc.gpsimd.memset(wsrc, 0.0)

            def guf(s, f):
                return gu_d[s][:, FC * f:FC * (f + 1)].rearrange(
                    "p (t k c) -> p t k c", t=2, k=KD)

            # The scalar engine must stay nearly DMA-free: its strict-FIFO
            # queue carries the ACT-table loads + every silu, and a queued
            # dma_start whose semaphore lane is still owned by an earlier
            # in-flight transfer blocks the whole queue (engine-side
            # sem-reuse wait) — which starves PSUM recycling and stalls
            # the PE. Scalar gets only 2 early critical pieces.
            # Measured queue behavior (run4 trace): early 3-way contention
            # gives each queue only ~82 GB/s; the SWDGE (gpsimd) queue
            # drains 2-3x faster than the HWDGE queues under contention.
            # mm1 of supertile 0 consumes one 0.4MB f-chunk per ~2.6us
            # (154 GB/s), more than any one queue — so consecutive
            # f-chunks must come from DIFFERENT queues, each queue's n-th
            # chunk aligned with the consumption cadence. Bulk mid-kernel
            # pieces (wd0, xt-rest, wd1) ride the fast gpsimd queue; gu1
            # rides sync ahead of the output stores.
            # sync queue
            nc.sync.dma_start(out=sc_sb, in_=sc_d[:, :])
            nc.sync.dma_start(out=xt_sb[:, :h0], in_=xt_d[:, :h0])
            for f in (4, 7, 10):
                nc.sync.dma_start(out=gu_sb[0][:, f], in_=guf(0, f))
            nc.sync.dma_start(
                out=wd_sb[0][:, :KF // 2],
                in_=wd_d[0][:, :KF * D // 2]
                .rearrange("p (k d) -> p k d", k=KF // 2))
            nc.sync.dma_start(
                out=gu_sb[1][:],
                in_=gu_d[1][:, :]
                .rearrange("p (f t k c) -> p f t k c", f=KF, t=2, k=KD))
            # scalar queue (early pieces only — its engine FIFO must be
            # free quickly for the ACT-table loads + the silu stream)
            nc.scalar.dma_start(out=xt_sb[:, h0:KD * ns0],
                                in_=xt_d[:, h0:KD * ns0])
            nc.scalar.dma_start(out=gu_sb[0][:, 1, 1],
                                in_=gu_d[0][:, FC + KD * P:2 * FC]
                                .rearrange("p (k c) -> p k c", k=KD))
            for f in (5, 8):
                nc.scalar.dma_start(out=gu_sb[0][:, f], in_=guf(0, f))
            # gpsimd queue
            # f0's gate half alone unblocks the first pg accumulation
            # (~0.2MB instead of 0.4MB in the critical set); the up half
            # follows ~1us later, before pu f0 needs it.
            nc.gpsimd.dma_start(out=gu_sb[0][:, 0, 0],
                                in_=gu_d[0][:, :KD * P]
                                .rearrange("p (k c) -> p k c", k=KD))
            nc.gpsimd.dma_start(out=gu_sb[0][:, 0, 1],
                                in_=gu_d[0][:, KD * P:FC]
                                .rearrange("p (k c) -> p k c", k=KD))
            nc.gpsimd.dma_start(out=gu_sb[0][:, 1, 0],
                                in_=gu_d[0][:, FC:FC + KD * P]
                                .rearrange("p (k c) -> p k c", k=KD))
            for f in (2, 3, 6, 9, 11):
                nc.gpsimd.dma_start(out=gu_sb[0][:, f], in_=guf(0, f))
            nc.gpsimd.dma_start(
                out=wd_sb[0][:, KF // 2:],
                in_=wd_d[0][:, KF * D // 2:]
                .rearrange("p (k d) -> p k d", k=KF // 2))
            nc.gpsimd.dma_start(out=xt_sb[:, KD * ns0:],
                                in_=xt_d[:, KD * ns0:])
            nc.gpsimd.dma_start(out=wd_sb[1][:], in_=wd_d[1][:, :]
                                .rearrange("p (k d) -> p k d", k=KF))

            # PE warm-up: the HAM clock gate holds the PE at 1.2GHz until
            # ~5us of sustained activity, and the critical DMAs land at
            # ~13us anyway. Burn the window on throwaway matmuls over the
            # zeroed tile so the real stream starts warm at 2.4GHz.
            wps = ps1.tile([P, 512], dt.float32, tag="pg")
            for i in range(N_WARM):
                nc.tensor.matmul(wps, wsrc[:, :P], wsrc,
                                 start=(i == 0), stop=(i == N_WARM - 1))

            # Execute any partial (pad-trimmed) routed tile LAST so the
            # epilogue after the final matmul is as short as possible.
            sts_exec = [t for t in sts if t[2] == t[1]] + \
                       [t for t in sts if t[2] != t[1]]
            for (r0, ns, nr, s) in sts_exec:
                h = work.tile([P, KF, nr], dt.bfloat16, tag="h")
                for f in range(KF):
                    pg = ps1.tile([P, nr], dt.float32, tag="pg")
                    pu = ps1.tile([P, nr], dt.float32, tag="pu")
                    for k in range(KD):
                        rhs = xt_sb[:, KD * r0 + k * ns: KD * r0 + k * ns + nr]
                        nc.tensor.matmul(pg, gu_sb[s][:, f, 0, k, :], rhs,
                                         start=(k == 0), stop=(k == KD - 1))
                    for k in range(KD):
                        rhs = xt_sb[:, KD * r0 + k * ns: KD * r0 + k * ns + nr]
                        nc.tensor.matmul(pu, gu_sb[s][:, f, 1, k, :], rhs,
                                         start=(k == 0), stop=(k == KD - 1))
                    sg = work.tile([P, nr], dt.float32, tag="sg")
                    nc.scalar.activation(sg, pg, silu)
                    nc.vector.tensor_mul(h[:, f, :], sg, pu)
                for sub in range((nr + P - 1) // P):
                    rt = r0 // P + sub
                    rows = min(P, nr - sub * P)
                    ss = slice(sub * P, sub * P + rows)
                    ob = outp.tile([P, D], dt.float32, tag="ob")
                    # po0's accumulation completes before po1's first
                    # matmul, so its scale + store overlap po1's group;
                    # the two 0.39MB half-stores also keep each output
                    # queue under its sustainable rate.
                    for half, po_tag in ((0, "po0"), (1, "po1")):
                        cs = slice(half * 384, half * 384 + 384)
                        po = ps2.tile([P, 384], dt.float32, tag=po_tag)
                        for f in range(KF):
                            nc.tensor.matmul(po[:rows], h[:, f, ss],
                                             wd_sb[s][:, f, cs],
                                             start=(f == 0),
                                             stop=(f == KF - 1))
                        nc.vector.tensor_scalar_mul(ob[:rows, cs], po[:rows],
                                                    sc_sb[:rows, rt:rt + 1])
                        # All stores go through the sync engine: it is idle
                        # after its input issues, while an out-issue on the
                        # scalar engine would block the next supertile's
                        # activations in its strict-FIFO queue.
                        nc.sync.dma_start(out=out_d[rt * P:rt * P + rows, cs],
                                          in_=ob[:rows, cs])

    nc.compile()
    return nc


def _get_program(r_total, c_pad, c_used):
    key = (r_total, c_pad, c_used)
    if key not in _prog_cache:
        _prog_cache[key] = _build_program(r_total, c_pad, c_used)
    return _prog_cache[key]


def prepare(x, Wg_s, Wu_s, Wd_s, Wg_r, Wu_r, Wd_r, W_router, expert_bias):
    """Host-side routing + sharding. Returns (nc, in_maps, assembly info)."""
    x = np.asarray(x, np.float32)
    B, S, _ = x.shape
    T = B * S
    sh = T // NCORES  # shared tokens per core
    xf = x.reshape(T, D)

    i1, i2, w1, w2 = _route_host(xf, np.asarray(W_router, np.float32),
                                 np.asarray(expert_bias, np.float32))

    tok_idx, tok_w = [], []
    for e in range(E):
        m1 = i1 == e
        m2 = i2 == e
        idx = np.concatenate([np.nonzero(m1)[0], np.nonzero(m2)[0]])
        w = np.concatenate([w1[m1], w2[m2]]).astype(np.float32)
        tok_idx.append(idx)
        tok_w.append(w)
    counts = [len(ix) for ix in tok_idx]
    c_pad = max(P, ((max(counts) + P - 1) // P) * P)
    r_total = c_pad + sh
    n_tiles = r_total // P
    sts = _supertiles(c_pad, r_total)

    xt_full = np.ascontiguousarray(xf.T.astype(BF16))  # [D, T]

    def wcast(a):
        return np.asarray(a, np.float32).astype(BF16)

    gu_s = _shuf_gu(wcast(Wg_s[0]), wcast(Wu_s[0]))
    wd_s = _shuf_wd(wcast(Wd_s[0]))
    in_maps = []
    for c in range(E):
        xt = np.zeros((D, r_total), BF16)
        xt[:, :counts[c]] = xt_full[:, tok_idx[c]]
        xt[:, c_pad:c_pad + sh] = xt_full[:, c * sh:(c + 1) * sh]
        scale = np.zeros(r_total, np.float32)
        scale[:counts[c]] = tok_w[c]
        scale[c_pad:c_pad + sh] = 1.0
        scale_t = np.ascontiguousarray(scale.reshape(n_tiles, P).T)
        in_maps.append({
            "xt": _shuf_xt(xt, sts),
            "gu0": _shuf_gu(wcast(Wg_r[c]), wcast(Wu_r[c])),
            "wd0": _shuf_wd(wcast(Wd_r[c])),
            "gu1": gu_s, "wd1": wd_s,
            "scale": scale_t,
        })

    nc = _get_program(r_total, c_pad, max(counts))
    info = dict(T=T, B=B, S=S, sh=sh, c_pad=c_pad, counts=counts,
                tok_idx=tok_idx)
    return nc, in_maps, info


def assemble(results, info):
    T, sh, c_pad = info["T"], info["sh"], info["c_pad"]
    out = np.zeros((T, D), np.float32)
    for c in range(NCORES):
        o = results[c]["out"]
        cnt = info["counts"][c]
        if cnt:
            out[info["tok_idx"][c]] += o[:cnt]
        out[c * sh:(c + 1) * sh] += o[c_pad:c_pad + sh]
    return out.reshape(info["B"], info["S"], D)


def kernel(x, Wg_s, Wu_s, Wd_s, Wg_r, Wu_r, Wd_r, W_router, expert_bias):
    nc, in_maps, info = prepare(x, Wg_s, Wu_s, Wd_s, Wg_r, Wu_r, Wd_r,
                                W_router, expert_bias)
    res = run_bass_kernel_spmd(nc, in_maps, list(range(NCORES)))
    return assemble(res.results, info)


# revision 31
# speedup vs baseline: 1.0119x; 1.0119x over previous
"""MoE FFN (shared + top-2 routed experts) on 8 Trainium2 NeuronCores.

Strategy (expert-parallel with host-side token dispatch):
  - Router (logits -> top-2 -> softmax) is computed on the host in float64;
    it is a tiny [T,768]@[768,8] matmul. The resulting combine weights and
    per-expert token lists define the dispatch.
  - Core c processes: (a) the tokens routed to expert c (gathered, padded to
    C_pad), through expert c's SwiGLU weights, scaled by combine weight;
    (b) a 512-token slice of all tokens through the shared expert.
  - Each core returns [C_pad + 512, 768] fp32 partial outputs; the host
    scatter-adds routed partials and adds the shared slices.

Device kernel (per core, SPMD — identical program, different data):
  The PE issue stream is the floor (~148us of matmuls at N/2.4GHz+2.5ns
  each); everything else is engineered so the PE never waits:
  - Gate/up weights are interleaved per 128-col f-chunk ([P, f, g|u, k, 128])
    so the critical set before the first real matmul is only ~1.2 MB
    (xt of supertile 0 + f-chunk 0), spread critical-first across the
    three DMA queues (sync/scalar HWDGE + gpsimd SWDGE). Supertile 0's
    mm1 consumes one 0.4MB f-chunk per ~2.6us; later supertiles reuse
    resident weights.
  - A short PE warm-up (zeros matmuls) spans the DMA window so the HAM
    clock gate is at 2.4GHz when real work starts (~12us).
  - mm1 computes H^T = (W.T)(X^T) per 128-wide F chunk accumulating over
    D in PSUM (fp32), ACT applies SiLU, DVE multiplies gate*up -> bf16.
    mm2 contracts over F with H^T chunks stationary, producing
    [128 rows, 768] fp32 in two 384-col PSUM banks; po0's accumulation
    completes before po1's starts and each half is scaled (per-row
    combine weight via per-partition tensor_scalar) and stored
    separately, so the final store overlaps the last matmul group.
  - The ragged supertile (rows beyond the last full 512) runs last to
    keep the post-matmul tail minimal.
"""

import numpy as np
import ml_dtypes

import concourse.bass as bass  # noqa: F401
import concourse.mybir as mybir
import concourse.tile as tile
from concourse import bacc
from concourse.bass_utils import run_bass_kernel_spmd

BF16 = ml_dtypes.bfloat16

D = 768
F = 1536
E = 8
K_ACTIVE = 2
NCORES = 8
P = 128
KD = D // P   # 6 contraction chunks for mm1
KF = F // P   # 12 contraction chunks for mm2 == f-chunks of mm1
N_WARM = 17   # PE warm-up matmuls. HAM empirically needs ~5us of
              # continuous PE-busy to un-throttle (not the nominal
              # 3.4us window), so the warm-up starts at ~6.4us (memset
              # on the early-free gpsimd engine, not vector) and runs
              # ~12 cold + ~5 warm matmuls, ending just as the critical
              # input DMAs land (~13us).

_prog_cache = {}


def _route_host(xf, w_router, expert_bias):
    """Top-2 routing in float64 (matches the fp32 jax reference to ~1e-7)."""
    T = xf.shape[0]
    logits = xf.astype(np.float64) @ w_router.astype(np.float64)
    logits += expert_bias.astype(np.float64)[None, :]
    ar = np.arange(T)
    i1 = np.argmax(logits, axis=1)
    v1 = logits[ar, i1]
    l2 = logits.copy()
    l2[ar, i1] = -np.inf
    i2 = np.argmax(l2, axis=1)
    v2 = logits[ar, i2]
    e2 = np.exp(v2 - v1)
    s = 1.0 + e2
    w1 = (1.0 / s).astype(np.float32)
    w2 = (e2 / s).astype(np.float32)
    return i1, i2, w1, w2


def _supertiles(c_pad, r_total):
    """Split rows into (start, size, weight_set) chunks of <=512 rows."""
    out = []
    r0 = 0
    while r0 < c_pad:
        ns = min(512, c_pad - r0)
        out.append((r0, ns, 0))
        r0 += ns
    while r0 < r_total:
        ns = min(512, r_total - r0)
        out.append((r0, ns, 1))
        r0 += ns
    return out


def _shuf_gu(wg, wu):
    """Two [D, F] mats -> [128, KF*2*KD*128]: partition-major, f-chunk-major
    interleaved layout; element [p, f, t, k, c] = W_t[k*128+p, f*128+c]."""
    def one(w):
        return w.reshape(KD, P, KF, P).transpose(1, 2, 0, 3)
    return np.ascontiguousarray(
        np.stack([one(wg), one(wu)], axis=2).reshape(P, KF * 2 * KD * P))


def _shuf_wd(w):
    """[F, D] -> [128, KF*D]: partition-major layout."""
    return np.ascontiguousarray(
        w.reshape(KF, P, D).transpose(1, 0, 2).reshape(P, KF * D))


def _shuf_xt(xt, sts):
    """[D, R] -> [128, KD*R] with per-supertile blocks."""
    out = np.empty((P, KD * xt.shape[1]), xt.dtype)
    for (r0, ns, _) in sts:
        blk = xt[:, r0:r0 + ns].reshape(KD, P, ns).transpose(1, 0, 2)
        out[:, KD * r0:KD * (r0 + ns)] = blk.reshape(P, KD * ns)
    return np.ascontiguousarray(out)


def _build_program(r_total, c_pad, c_used):
    dt = mybir.dt
    nc = bacc.Bacc("TRN2", target_bir_lowering=False, debug=False)
    xt_d = nc.dram_tensor("xt", [P, KD * r_total], dt.bfloat16,
                          kind="ExternalInput")
    gu_d = [nc.dram_tensor(f"gu{s}", [P, KF * 2 * KD * P], dt.bfloat16,
                           kind="ExternalInput") for s in range(2)]
    wd_d = [nc.dram_tensor(f"wd{s}", [P, KF * D], dt.bfloat16,
                           kind="ExternalInput") for s in range(2)]
    n_tiles = r_total // P
    sc_d = nc.dram_tensor("scale", [P, n_tiles], dt.float32,
                          kind="ExternalInput")
    out_d = nc.dram_tensor("out", [r_total, D], dt.float32,
                           kind="ExternalOutput")

    silu = mybir.ActivationFunctionType.Silu
    # (row0, layout rows, compute rows, weight set): pad rows beyond the
    # actual max routed count exist in the layout but are never computed.
    sts = []
    for (r0, ns, s) in _supertiles(c_pad, r_total):
        nc_rows = ns if s == 1 else min(ns, max(0, c_used - r0))
        if nc_rows > 0:
            sts.append((r0, ns, nc_rows, s))

    FC = 2 * KD * P  # elements per f-chunk of a gu tensor (per partition)

    with tile.TileContext(nc) as tc:
        with (
            tc.tile_pool(name="const", bufs=1) as const,
            tc.tile_pool(name="work", bufs=3) as work,
            tc.tile_pool(name="outp", bufs=8) as outp,
            tc.tile_pool(name="ps1", bufs=2, space="PSUM") as ps1,
            tc.tile_pool(name="ps2", bufs=2, space="PSUM") as ps2,
        ):
            sc_sb = const.tile([P, n_tiles], dt.float32, tag="sc")
            xt_sb = const.tile([P, KD * r_total], dt.bfloat16, tag="xt")
            gu_sb, wd_sb = [], []
            for s in range(2):
                gu_t = const.tile([P, KF, 2, KD, P], dt.bfloat16,
                                  tag=f"gu{s}")
                wd_t = const.tile([P, KF, D], dt.bfloat16, tag=f"wd{s}")
                gu_sb.append(gu_t)
                wd_sb.append(wd_t)

            # DMA plan, critical-first. The first real matmul needs
            # xt(supertile 0) + gu0 f-chunk 0 (~1.2 MB); each further
            # f-chunk (0.4 MB) is consumed one per ~2.6us. Queues drain
            # FIFO per engine with round-robin across queues (~1/3 of
            # ~320 GB/s each while all are busy), so the critical pieces
            # lead all three queues and f-chunks rotate across them.
            r0_, ns0 = sts[0][0], sts[0][1]
            h0 = KD * ns0 // 2

            # Warm-up source tile, zeroed on gpsimd BEFORE its dma_start
            # instructions (each ~1us of SWDGE emission) so the PE can
            # begin warming at ~6.4us; a vector-engine memset would wait
            # out the longer vector preamble and delay warm-up to ~8.4us.
            wsrc = const.tile([P, 512], dt.bfloat16, tag="wsrc")
            nc.gpsimd.memset(wsrc, 0.0)

            def guf(s, f):
                return gu_d[s][:, FC * f:FC * (f + 1)].rearrange(
                    "p (t k c) -> p t k c", t=2, k=KD)

            # The scalar engine must stay nearly DMA-free: its strict-FIFO
            # queue carries the ACT-table loads + every silu, and a queued
            # dma_start whose semaphore lane is still owned by an earlier
            # in-flight transfer blocks the whole queue (engine-side
            # sem-reuse wait) — which starves PSUM recycling and stalls
            # the PE. Scalar gets only 2 early critical pieces.
            # Measured queue behavior (run4 trace): early 3-way contention
            # gives each queue only ~82 GB/s; the SWDGE (gpsimd) queue
            # drains 2-3x faster than the HWDGE queues under contention.
            # mm1 of supertile 0 consumes one 0.4MB f-chunk per ~2.6us
            # (154 GB/s), more than any one queue — so consecutive
            # f-chunks must come from DIFFERENT queues, each queue's n-th
            # chunk aligned with the consumption cadence. Bulk mid-kernel
            # pieces (wd0, xt-rest, wd1) ride the fast gpsimd queue; gu1
            # rides sync ahead of the output stores.
            # sync queue
            nc.sync.dma_start(out=sc_sb, in_=sc_d[:, :])
            nc.sync.dma_start(out=xt_sb[:, :h0], in_=xt_d[:, :h0])
            for f in (4, 7, 10):
                nc.sync.dma_start(out=gu_sb[0][:, f], in_=guf(0, f))
            # wd0 in f-major thirds, one per queue: mm2 reads f in order,
            # so it can start on the first third, and each third has
            # >=10us of arrival margin against per-core queue variance
            # (a late whole-wd0 cost core 3 a 4.4us stall + HAM rethrottle).
            nc.sync.dma_start(
                out=wd_sb[0][:, 2 * KF // 3:],
                in_=wd_d[0][:, 2 * KF * D // 3:]
                .rearrange("p (k d) -> p k d", k=KF // 3))
            nc.sync.dma_start(
                out=gu_sb[1][:],
                in_=gu_d[1][:, :]
                .rearrange("p (f t k c) -> p f t k c", f=KF, t=2, k=KD))
            # scalar queue (early pieces only — its engine FIFO must be
            # free quickly for the ACT-table loads + the silu stream)
            nc.scalar.dma_start(out=xt_sb[:, h0:KD * ns0],
                                in_=xt_d[:, h0:KD * ns0])
            nc.scalar.dma_start(out=gu_sb[0][:, 1, 1],
                                in_=gu_d[0][:, FC + KD * P:2 * FC]
                                .rearrange("p (k c) -> p k c", k=KD))
            for f in (5, 8):
                nc.scalar.dma_start(out=gu_sb[0][:, f], in_=guf(0, f))
            nc.scalar.dma_start(
                out=wd_sb[0][:, :KF // 3],
                in_=wd_d[0][:, :KF * D // 3]
                .rearrange("p (k d) -> p k d", k=KF // 3))
            # gpsimd queue
            # f0's gate half alone unblocks the first pg accumulation
            # (~0.2MB instead of 0.4MB in the critical set); the up half
            # follows ~1us later, before pu f0 needs it.
            nc.gpsimd.dma_start(out=gu_sb[0][:, 0, 0],
                                in_=gu_d[0][:, :KD * P]
                                .rearrange("p (k c) -> p k c", k=KD))
            nc.gpsimd.dma_start(out=gu_sb[0][:, 0, 1],
                                in_=gu_d[0][:, KD * P:FC]
                                .rearrange("p (k c) -> p k c", k=KD))
            nc.gpsimd.dma_start(out=gu_sb[0][:, 1, 0],
                                in_=gu_d[0][:, FC:FC + KD * P]
                                .rearrange("p (k c) -> p k c", k=KD))
            for f in (2, 3, 6, 9, 11):
                nc.gpsimd.dma_start(out=gu_sb[0][:, f], in_=guf(0, f))
            nc.gpsimd.dma_start(
                out=wd_sb[0][:, KF // 3:2 * KF // 3],
                in_=wd_d[0][:, KF * D // 3:2 * KF * D // 3]
                .rearrange("p (k d) -> p k d", k=KF // 3))
            nc.gpsimd.dma_start(out=xt_sb[:, KD * ns0:],
                                in_=xt_d[:, KD * ns0:])
            nc.gpsimd.dma_start(out=wd_sb[1][:], in_=wd_d[1][:, :]
                                .rearrange("p (k d) -> p k d", k=KF))

            # PE warm-up: the HAM clock gate holds the PE at 1.2GHz until
            # ~5us of sustained activity, and the critical DMAs land at
            # ~13us anyway. Burn the window on throwaway matmuls over the
            # zeroed tile so the real stream starts warm at 2.4GHz.
            wps = ps1.tile([P, 512], dt.float32, tag="pg")
            for i in range(N_WARM):
                nc.tensor.matmul(wps, wsrc[:, :P], wsrc,
                                 start=(i == 0), stop=(i == N_WARM - 1))

            # Execute any partial (pad-trimmed) routed tile LAST so the
            # epilogue after the final matmul is as short as possible.
            sts_exec = [t for t in sts if t[2] == t[1]] + \
                       [t for t in sts if t[2] != t[1]]
            for (r0, ns, nr, s) in sts_exec:
                h = work.tile([P, KF, nr], dt.bfloat16, tag="h")
                for f in range(KF):
                    pg = ps1.tile([P, nr], dt.float32, tag="pg")
                    pu = ps1.tile([P, nr], dt.float32, tag="pu")
                    for k in range(KD):
                        rhs = xt_sb[:, KD * r0 + k * ns: KD * r0 + k * ns + nr]
                        nc.tensor.matmul(pg, gu_sb[s][:, f, 0, k, :], rhs,
                                         start=(k == 0), stop=(k == KD - 1))
                    for k in range(KD):
                        rhs = xt_sb[:, KD * r0 + k * ns: KD * r0 + k * ns + nr]
                        nc.tensor.matmul(pu, gu_sb[s][:, f, 1, k, :], rhs,
                                         start=(k == 0), stop=(k == KD - 1))
                    sg = work.tile([P, nr], dt.float32, tag="sg")
                    nc.scalar.activation(sg, pg, silu)
                    nc.vector.tensor_mul(h[:, f, :], sg, pu)
                for sub in range((nr + P - 1) // P):
                    rt = r0 // P + sub
                    rows = min(P, nr - sub * P)
                    ss = slice(sub * P, sub * P + rows)
                    ob = outp.tile([P, D], dt.float32, tag="ob")
                    # po0's accumulation completes before po1's first
                    # matmul, so its scale + store overlap po1's group;
                    # the two 0.39MB half-stores also keep each output
                    # queue under its sustainable rate.
                    for half, po_tag in ((0, "po0"), (1, "po1")):
                        cs = slice(half * 384, half * 384 + 384)
                        po = ps2.tile([P, 384], dt.float32, tag=po_tag)
                        for f in range(KF):
                            nc.tensor.matmul(po[:rows], h[:, f, ss],
                                             wd_sb[s][:, f, cs],
                                             start=(f == 0),
                                             stop=(f == KF - 1))
                        nc.vector.tensor_scalar_mul(ob[:rows, cs], po[:rows],
                                                    sc_sb[:rows, rt:rt + 1])
                        # All stores go through the sync engine: it is idle
                        # after its input issues, while an out-issue on the
                        # scalar engine would block the next supertile's
                        # activations in its strict-FIFO queue.
                        nc.sync.dma_start(out=out_d[rt * P:rt * P + rows, cs],
                                          in_=ob[:rows, cs])

    nc.compile()
    return nc


def _get_program(r_total, c_pad, c_used):
    key = (r_total, c_pad, c_used)
    if key not in _prog_cache:
        _prog_cache[key] = _build_program(r_total, c_pad, c_used)
    return _prog_cache[key]


def prepare(x, Wg_s, Wu_s, Wd_s, Wg_r, Wu_r, Wd_r, W_router, expert_bias):
    """Host-side routing + sharding. Returns (nc, in_maps, assembly info)."""
    x = np.asarray(x, np.float32)
    B, S, _ = x.shape
    T = B * S
    sh = T // NCORES  # shared tokens per core
    xf = x.reshape(T, D)

    i1, i2, w1, w2 = _route_host(xf, np.asarray(W_router, np.float32),
                                 np.asarray(expert_bias, np.float32))

    tok_idx, tok_w = [], []
    for e in range(E):
        m1 = i1 == e
        m2 = i2 == e
        idx = np.concatenate([np.nonzero(m1)[0], np.nonzero(m2)[0]])
        w = np.concatenate([w1[m1], w2[m2]]).astype(np.float32)
        tok_idx.append(idx)
        tok_w.append(w)
    counts = [len(ix) for ix in tok_idx]
    c_pad = max(P, ((max(counts) + P - 1) // P) * P)
    r_total = c_pad + sh
    n_tiles = r_total // P
    sts = _supertiles(c_pad, r_total)

    xt_full = np.ascontiguousarray(xf.T.astype(BF16))  # [D, T]

    def wcast(a):
        return np.asarray(a, np.float32).astype(BF16)

    gu_s = _shuf_gu(wcast(Wg_s[0]), wcast(Wu_s[0]))
    wd_s = _shuf_wd(wcast(Wd_s[0]))
    in_maps = []
    for c in range(E):
        xt = np.zeros((D, r_total), BF16)
        xt[:, :counts[c]] = xt_full[:, tok_idx[c]]
        xt[:, c_pad:c_pad + sh] = xt_full[:, c * sh:(c + 1) * sh]
        scale = np.zeros(r_total, np.float32)
        scale[:counts[c]] = tok_w[c]
        scale[c_pad:c_pad + sh] = 1.0
        scale_t = np.ascontiguousarray(scale.reshape(n_tiles, P).T)
        in_maps.append({
            "xt": _shuf_xt(xt, sts),
            "gu0": _shuf_gu(wcast(Wg_r[c]), wcast(Wu_r[c])),
            "wd0": _shuf_wd(wcast(Wd_r[c])),
            "gu1": gu_s, "wd1": wd_s,
            "scale": scale_t,
        })

    nc = _get_program(r_total, c_pad, max(counts))
    info = dict(T=T, B=B, S=S, sh=sh, c_pad=c_pad, counts=counts,
                tok_idx=tok_idx)
    return nc, in_maps, info


def assemble(results, info):
    T, sh, c_pad = info["T"], info["sh"], info["c_pad"]
    out = np.zeros((T, D), np.float32)
    for c in range(NCORES):
        o = results[c]["out"]
        cnt = info["counts"][c]
        if cnt:
            out[info["tok_idx"][c]] += o[:cnt]
        out[c * sh:(c + 1) * sh] += o[c_pad:c_pad + sh]
    return out.reshape(info["B"], info["S"], D)


def kernel(x, Wg_s, Wu_s, Wd_s, Wg_r, Wu_r, Wd_r, W_router, expert_bias):
    nc, in_maps, info = prepare(x, Wg_s, Wu_s, Wd_s, Wg_r, Wu_r, Wd_r,
                                W_router, expert_bias)
    res = run_bass_kernel_spmd(nc, in_maps, list(range(NCORES)))
    return assemble(res.results, info)
